# revision 25
# baseline (speedup 1.0000x reference)
"""BarMamba sparse-attention kernel for Trainium2 (8 NeuronCores, data-parallel over batch).

Stage 1: block-diagonal bar attention (bar token attends to its 31 notes).
Stage 2: DSNT attention of bar summaries over memory level 2 (W=1024).

Design notes (per core, batch element b):
 - Everything is matmul-centric. Activations live in "transposed" layouts
   (D on partitions) so the PE can contract over D; fp32 data is bitcast to
   float32r for full-rate matmuls (free dim >= 256).
 - Stage-1 scores are computed per head as 4 column-tiled matmuls
   (32 bars each) over that group's 992 packed note-keys, with a
   block-diagonal -1e9 mask added via an extra identity matmul; exp on ACT
   produces unnormalized weights and their row sums (softmax denominators)
   in one instruction.
 - ctx = aw @ V runs on the PE via per-(head, group) accumulated matmuls
   with PE-transposed attention weights.
 - Softmax normalization is folded into the ctx PSUM eviction.
 - Stage-2 softmax over W=1024 is unnormalized-exp + fused dot with t_norm
   (scalar_tensor_tensor accumulate), normalized at the end.
"""

import math
from contextlib import ExitStack

import numpy as np
import ml_dtypes

import concourse.bass as bass
import concourse.bacc as bacc
import concourse.mybir as mybir
import concourse.tile as tile
from concourse.vector_clock import ScopedClock
from concourse.bass import SemaphoreHandle
from concourse.bass_utils import run_bass_kernel_spmd

B, S, D, H, P = 8, 4096, 512, 8, 32
NBAR = S // P        # 128 bars
DH = D // H          # 64
NOTE = P - 1         # 31 notes per bar
GRP = 32             # bars per score column-group
NGRP = NBAR // GRP   # 4
KEYS = GRP * P       # 1024 keys per group (bar token = masked dummy key)
NTILE = 128          # keys per 128-token window (4 bars x 32)
W = 1024             # level-2 width
LVL = 2
NEG = -1.0e9

F32 = mybir.dt.float32
F32R = mybir.dt.float32r
BF16 = mybir.dt.bfloat16
AX = mybir.AxisListType
ALU = mybir.AluOpType
ACTF = mybir.ActivationFunctionType


class PatchedTileContext(tile.TileContext):
    """TileContext whose exit drain splits sem waits into standalone wait
    instructions (this walrus build caps sync-waits per CTRL instruction)."""

    def _drain_and_barrier(self, tick_clock, wait_clock):
        nc = self.nc
        probe = mybir.InstDrain(name=f"I-probe-{nc.next_id()}")
        probe.engine = mybir.EngineType.SP
        wait_clock.add_sem_waits(probe, ScopedClock({None: tick_clock.global_clock}))
        si = probe.sync_info
        waits = list(si.on_wait) if si is not None else []
        for w in waits:
            sem = SemaphoreHandle(num=w.id, name=w.ant_name)
            nc.sync.wait_ge(sem, w.wait_value)
        nc.sync.drain()
        nc.all_engine_barrier()
        assert self.sems is not None
        popped = nc._tile_sem_poison_stack.pop()
        assert popped is self._sem_poison
        nc.clear_and_free_semaphores(list(self.sems.allocated().values()))
        nc.all_engine_barrier()


def r(ap):
    return ap.bitcast(F32R)


def _build():
    nc = bacc.Bacc()

    y_ext = nc.declare_dram_parameter("y", [S, D], F32R, isOutput=False)
    mem_ext = nc.declare_dram_parameter("mem", [W, D], F32R, isOutput=False)
    wnames = ["wq", "wk", "wv", "wo", "wq2", "wkm"]
    w_ext = {n: nc.declare_dram_parameter(n, [128, 4, D], F32R, isOutput=False)
             for n in wnames}
    peT_ext = nc.declare_dram_parameter("peT", [128, 4, NBAR], F32, isOutput=False)
    tpeT_ext = nc.declare_dram_parameter("tpeT", [128, 4, W], BF16, isOutput=False)
    tbc_ext = nc.declare_dram_parameter("tbc", [128, W], F32, isOutput=False)
    maskR_ext = nc.declare_dram_parameter("maskR", [128, KEYS], F32R, isOutput=False)
    id_ext = nc.declare_dram_parameter("ident", [128, 128], F32R, isOutput=False)
    idb_ext = nc.declare_dram_parameter("identb", [128, 128], BF16, isOutput=False)
    out_sum = nc.declare_dram_parameter("out_sum", [S, D], F32, isOutput=True)
    out_com = nc.declare_dram_parameter("out_com", [S, 1], F32, isOutput=True)

    with tile.TileContext(nc) as tc, ExitStack() as ctx:
        cpool = ctx.enter_context(tc.tile_pool(name="consts", bufs=1))
        small = ctx.enter_context(tc.tile_pool(name="small", bufs=1))
        pA = ctx.enter_context(tc.tile_pool(name="pA", bufs=3, space="PSUM"))
        pB = ctx.enter_context(tc.tile_pool(name="pB", bufs=2, space="PSUM"))
        pC = ctx.enter_context(tc.tile_pool(name="pC", bufs=1, space="PSUM"))

        # ---- constants / weights ----
        ident = cpool.tile([128, 128], F32R, tag="ident")
        nc.sync.dma_start(ident[:], id_ext[:])
        identb = cpool.tile([128, 128], BF16, tag="identb")
        nc.sync.dma_start(identb[:], idb_ext[:])
        peT = cpool.tile([128, 4, NBAR], F32, tag="peT")
        nc.sync.dma_start(peT[:], peT_ext[:])
        maskR = cpool.tile([128, KEYS], F32R, tag="maskR")
        wsb = {}
        for n in ["wq", "wk", "wv"]:
            wsb[n] = cpool.tile([128, 4, D], F32R, tag=f"w_{n}", name=f"w_{n}")

        denom1 = small.tile([128, H], F32, tag="denom1")
        denom2 = small.tile([128, H], F32, tag="denom2")
        ctraw = small.tile([128, H], F32, tag="ctraw")
        awt = small.tile([128, H, 8, NBAR], BF16, tag="awt")
        summary = small.tile([NBAR, D], F32R, tag="summary")
        com = small.tile([NBAR, 1], F32, tag="com")

        def transpose4(src, tag):
            """src: [128, 512] f32 sbuf -> psum [128, 4, 128] with
            pt[p, dt, c] = src[c, dt*128 + p]."""
            pt = pA.tile([128, 4, 128], F32, tag="pa")
            for dt in range(4):
                nc.tensor.transpose(r(pt[:, dt, :]), r(src[:, dt * 128:(dt + 1) * 128]),
                                    r(ident[:]))
            return pt

        vtp = ctx.enter_context(tc.tile_pool(name="vtp", bufs=1))
        ytp_ctx = tc.tile_pool(name="ytp", bufs=1, side="right")
        ytp = ytp_ctx.__enter__()
        with tc.tile_pool(name="stage1", bufs=1) as s1, \
             tc.tile_pool(name="yin", bufs=3) as yin, \
             tc.tile_pool(name="awp", bufs=2) as awp:

            yt = ytp.tile([128, 4, S], F32R, tag="yt")    # y_pe^T  (8 MiB)
            vt = vtp.tile([128, 32, D], BF16, tag="vt")   # V tiles (4 MiB)
            kt = s1.tile([128, 4, S], BF16, tag="kt")     # K^T     (4 MiB)

            # ---- phase A: load y, transpose(+bar PE), project V and K ----
            for t in range(32):
                ytile = yin.tile([128, D], F32R, tag="ytile")
                nc.sync.dma_start(ytile[:], y_ext[t * 128:(t + 1) * 128, :])
                if t == 0:
                    nc.sync.dma_start(wsb["wv"][:], w_ext["wv"][:])
                elif t == 1:
                    nc.sync.dma_start(wsb["wk"][:], w_ext["wk"][:])
                elif t == 8:
                    nc.sync.dma_start(wsb["wq"][:], w_ext["wq"][:])
                    nc.sync.dma_start(maskR[:], maskR_ext[:])
                pt = transpose4(ytile, "ptrans")
                o = yt[:, :, t * 128:(t + 1) * 128].rearrange(
                    "p dt (j q) -> p dt j q", q=P)
                i0 = pt[:].rearrange("p dt (j q) -> p dt j q", q=P)
                i1 = peT[:, :, 4 * t:4 * t + 4].broadcast_to((128, 4, 4, P))
                nc.vector.tensor_add(o, i0, i1)

                pv = pA.tile([128, 512], F32, tag="pa")
                for kti in range(4):
                    lhsT = yt[:, kti, t * 128:(t + 1) * 128]
                    nc.tensor.matmul(pv[:], r(lhsT), r(wsb["wv"][:, kti, :]),
                                     start=(kti == 0), stop=(kti == 3))
                if t % 2 == 0:
                    nc.scalar.copy(vt[:, t, :], pv[:])
                else:
                    nc.vector.tensor_copy(vt[:, t, :], pv[:])

                if t % 4 == 3:
                    c = t // 4
                    for mt in range(4):
                        pk = pA.tile([128, 512], F32, tag="pa")
                        for kti in range(4):
                            nc.tensor.matmul(
                                pk[:], r(wsb["wk"][:, kti, mt * 128:(mt + 1) * 128]),
                                r(yt[:, kti, c * 512:(c + 1) * 512]),
                                start=(kti == 0), stop=(kti == 3))
                        dst = kt[:, mt, c * 512:(c + 1) * 512]
                        if mt % 2 == 0:
                            nc.scalar.copy(dst, pk[:])
                        else:
                            nc.vector.tensor_copy(dst, pk[:])

            # ---- phase q: bar-token queries, then transpose ----
            pq = pA.tile([128, D], F32, tag="pa")
            for kti in range(4):
                bars = yt[:, kti, :].rearrange("p (n q) -> p n q", q=P)[:, :, 0:1]
                nc.tensor.matmul(pq[:], r(bars), r(wsb["wq"][:, kti, :]),
                                 start=(kti == 0), stop=(kti == 3))
            q_sb = small.tile([128, D], F32R, tag="scr512")
            nc.vector.tensor_copy(q_sb[:], pq[:])
            pt = transpose4(q_sb, "ptrans")
            qt = small.tile([128, 4, 128], BF16, tag="qt")
            nc.vector.tensor_copy(qt[:], pt[:])

            ytp_ctx.__exit__(None, None, None)

            # ---- prefetch stage-2 inputs + weights while scores run ----
            s2 = ctx.enter_context(tc.tile_pool(name="s2", bufs=1, side="right"))
            minp = ctx.enter_context(tc.tile_pool(name="minp", bufs=2, side="right"))
            late = ctx.enter_context(tc.tile_pool(name="late", bufs=1, side="right"))
            for n in ["wo", "wq2", "wkm"]:
                wsb[n] = late.tile([128, 4, D], F32R, tag=f"w_{n}", name=f"w_{n}")
                nc.sync.dma_start(wsb[n][:], w_ext[n][:])
            tpeT = s2.tile([128, 4, W], BF16, tag="tpeT")
            nc.sync.dma_start(tpeT[:], tpeT_ext[:])
            tbc = s2.tile([128, W], F32, tag="tbc")
            nc.sync.dma_start(tbc[:], tbc_ext[:])
            memT = s2.tile([128, 4, W], F32R, tag="memT")
            kmemT = s2.tile([128, 4, W], F32R, tag="kmemT")
            for t in range(8):
                mtile = minp.tile([128, D], F32R, tag="mtile")
                nc.sync.dma_start(mtile[:], mem_ext[t * 128:(t + 1) * 128, :])
                pt = transpose4(mtile, "ptrans")
                nc.vector.tensor_copy(memT[:, :, t * 128:(t + 1) * 128], pt[:])
            for mt in range(4):
                for wc in range(2):
                    pk = pA.tile([128, 512], F32, tag="pa")
                    for kti in range(4):
                        nc.tensor.matmul(
                            pk[:], r(wsb["wkm"][:, kti, mt * 128:(mt + 1) * 128]),
                            r(memT[:, kti, wc * 512:(wc + 1) * 512]),
                            start=(kti == 0), stop=(kti == 3))
                    nc.vector.tensor_add(
                        kmemT[:, mt, wc * 512:(wc + 1) * 512], pk[:],
                        tpeT[:, mt, wc * 512:(wc + 1) * 512])

            # ---- scores + exp + transposed weights, per head ----
            for h in range(H):
                mt = h // 2
                p0 = (h % 2) * DH
                ps = pB.tile([128, KEYS], F32, tag="pb")
                for wc in range(2):
                    nc.tensor.matmul(ps[:, wc * 512:(wc + 1) * 512],
                                     r(ident[:]),
                                     r(maskR[:, wc * 512:(wc + 1) * 512]),
                                     start=True, stop=False,
                                     skip_group_check=True)
                for g in range(NGRP):
                    lhsT = qt[p0:p0 + DH, mt, g * GRP:(g + 1) * GRP]
                    for wc in range(2):
                        rhs = kt[p0:p0 + DH, mt,
                                 g * KEYS + wc * 512:g * KEYS + (wc + 1) * 512]
                        nc.tensor.matmul(
                            ps[g * GRP:(g + 1) * GRP, wc * 512:(wc + 1) * 512],
                            lhsT, rhs,
                            start=False, stop=True,
                            skip_group_check=True,
                            tile_position=(p0, g * GRP))
                aw = awp.tile([128, KEYS], BF16, tag="aw")
                nc.scalar.activation(aw[:], ps[:], ACTF.Exp, bias=0.0,
                                     scale=0.125,
                                     accum_out=denom1[:, h:h + 1])
                paw = pB.tile([128, 8, 128], BF16, tag="pb")
                for ct in range(8):
                    nc.tensor.transpose(paw[:, ct, :],
                                        aw[:, ct * 128:(ct + 1) * 128],
                                        identb[:])
                nc.scalar.copy(awt[:, h, :, :], paw[:])

        # ---- phase ctx: PE-accumulated attention readout ----
        with tc.tile_pool(name="ctxp", bufs=1) as ctxp:
            psc = pC.tile([128, D], F32, tag="pctx")
            for h in range(H):
                for g in range(NGRP):
                    for ct in range(8):
                        lhsT = awt[:, h, ct, g * GRP:(g + 1) * GRP]
                        rhs = vt[:, 8 * g + ct, h * DH:(h + 1) * DH]
                        nc.tensor.matmul(
                            psc[g * GRP:(g + 1) * GRP, h * DH:(h + 1) * DH],
                            lhsT, rhs, start=(ct == 0), stop=(ct == 7),
                            skip_group_check=True, tile_position=(0, g * GRP))
            recip1 = small.tile([128, H], F32, tag="recip1")
            nc.vector.reciprocal(recip1[:], denom1[:])
            ctxn = small.tile([128, D], F32R, tag="scr512")
            nc.vector.tensor_mul(
                ctxn[:].rearrange("p (h d) -> p h d", h=H),
                psc[:].rearrange("p (h d) -> p h d", h=H),
                recip1[:].broadcast_to((128, H, DH)))

            # ---- summary = ctxn @ Wo ----
            pt = transpose4(ctxn, "ptrans")
            ctxT = small.tile([128, 4, 128], F32R, tag="ctxT")
            nc.vector.tensor_copy(ctxT[:], pt[:])
            psu = pA.tile([128, D], F32, tag="pa")
            for kt in range(4):
                nc.tensor.matmul(psu[:], r(ctxT[:, kt, :]), r(wsb["wo"][:, kt, :]),
                                 start=(kt == 0), stop=(kt == 3))
            nc.vector.tensor_copy(summary[:], psu[:])
            z32 = small.tile([32, D], F32, tag="z32")
            nc.gpsimd.memset(z32[:], 0.0)
            nc.sync.dma_start(out_sum[0:P, :], z32[:])
            nc.sync.dma_start(
                out_sum[P:S, :].rearrange("(j q) d -> j q d", q=P),
                summary[0:NBAR - 1, :].bitcast(F32).unsqueeze(1)
                .broadcast_to((NBAR - 1, P, D)))

            # ---- Q2 = summary @ Wq2, then transpose ----
            pt = transpose4(summary, "ptrans")
            sumT = small.tile([128, 4, 128], F32R, tag="sumT")
            nc.vector.tensor_copy(sumT[:], pt[:])
            pq2 = pA.tile([128, D], F32, tag="pa")
            for kt in range(4):
                nc.tensor.matmul(pq2[:], r(sumT[:, kt, :]), r(wsb["wq2"][:, kt, :]),
                                 start=(kt == 0), stop=(kt == 3))
            q2_sb = small.tile([128, D], F32R, tag="scr512")
            nc.vector.tensor_copy(q2_sb[:], pq2[:])
            pt = transpose4(q2_sb, "ptrans")
            q2t = small.tile([128, 4, 128], F32R, tag="q2t")
            nc.vector.tensor_copy(q2t[:], pt[:])

        # ---- stage 2: memory attention ----
        with tc.tile_pool(name="e2p", bufs=2) as e2p:
            for h in range(H):
                p0 = (h % 2) * DH
                mt = h // 2
                ps2 = pB.tile([128, W], F32, tag="pb")
                for wc in range(2):
                    nc.tensor.matmul(
                        ps2[:, wc * 512:(wc + 1) * 512],
                        r(q2t[p0:p0 + DH, mt, :]),
                        r(kmemT[p0:p0 + DH, mt, wc * 512:(wc + 1) * 512]),
                        start=True, stop=True)
                e2 = e2p.tile([128, W], F32, tag="e2")
                nc.scalar.activation(e2[:], ps2[:], ACTF.Exp, bias=0.0, scale=0.125,
                                     accum_out=denom2[:, h:h + 1])
                scr = e2p.tile([128, W], F32, tag="scr")
                nc.vector.scalar_tensor_tensor(
                    out=scr[:], in0=e2[:], scalar=1.0, in1=tbc[:],
                    op0=ALU.mult, op1=ALU.mult,
                    accum_out=ctraw[:, h:h + 1])

            # com = (1/H) * sum_h ctraw/denom2
            recip2 = small.tile([128, H], F32, tag="recip2")
            nc.vector.reciprocal(recip2[:], denom2[:])
            tmp8 = small.tile([128, H], F32, tag="tmp8")
            nc.vector.tensor_mul(tmp8[:], ctraw[:], recip2[:])
            nc.vector.tensor_reduce(com[:], tmp8[:], axis=AX.X, op=ALU.add)
            nc.scalar.mul(com[:], com[:], 1.0 / H)

        # ---- com output ----
        with tc.tile_pool(name="outp", bufs=1) as outp:
            T = outp.tile([128, P], F32, tag="Tcom")
            nc.gpsimd.memset(T[:], 0.0)
            nc.sync.dma_start(T[1:128, 0:1], com[0:NBAR - 1, 0:1])
            nc.sync.dma_start(out_com[:].rearrange("(j q) o -> j q o", q=P),
                              T[:].unsqueeze(-1))
    nc.compile()
    return nc


def _sin_pe_np(pos, d):
    half = d // 2
    inv_freq = (1.0 / (10000.0 ** (np.arange(half, dtype=np.float32) / half))
                ).astype(np.float32)
    ang = pos.astype(np.float32)[:, None] * inv_freq[None, :]
    return np.concatenate([np.sin(ang), np.cos(ang)], axis=-1).astype(np.float32)


_NC_CACHE = {}


def kernel(y, memory, spatial_shapes, level_start_index, bar_mask,
           Wq_bar, Wk_bar, Wv_bar, Wo_bar, Wq2, Wk_mem, _trace=False):
    y = np.asarray(y)
    memory = np.asarray(memory)
    bar_mask = np.asarray(bar_mask)
    start = int(np.asarray(level_start_index)[LVL])
    w_l = int(np.asarray(spatial_shapes)[LVL, 1])
    assert w_l == W and y.shape == (B, S, D)

    # bar PE table (bar_mask is periodic with period P; PE constant per block)
    bidx = np.cumsum(bar_mask[0].astype(np.float32))[::P]          # [128]
    peJ = _sin_pe_np(bidx, D)                                      # [128, 512]
    peT = np.ascontiguousarray(peJ.reshape(NBAR, 4, 128).transpose(2, 1, 0))

    t_norm = (np.arange(W, dtype=np.float32) / max(W - 1, 1)).astype(np.float32)
    time_pe = _sin_pe_np(t_norm * W, D)                            # [1024, 512]
    tpeT = np.ascontiguousarray(time_pe.reshape(W, 4, 128).transpose(2, 1, 0)).astype(ml_dtypes.bfloat16)
    tbc = np.ascontiguousarray(np.broadcast_to(t_norm, (128, W)))

    key_idx = np.arange(KEYS)
    key_grp = key_idx // P
    key_k = key_idx % P
    diag = (key_grp[None, :] == (np.arange(128)[:, None] % GRP)) & (key_k[None, :] > 0)
    maskR = np.where(diag, 0.0, NEG).astype(np.float32)
    ident = np.eye(128, dtype=np.float32)
    identb = np.eye(128).astype(ml_dtypes.bfloat16)

    def wprep(w):
        return np.ascontiguousarray(
            np.asarray(w, dtype=np.float32).reshape(4, 128, D).transpose(1, 0, 2))

    shared = {
        "wq": wprep(Wq_bar), "wk": wprep(Wk_bar), "wv": wprep(Wv_bar),
        "wo": wprep(Wo_bar), "wq2": wprep(Wq2), "wkm": wprep(Wk_mem),
        "peT": peT, "tpeT": tpeT, "tbc": tbc, "maskR": maskR,
        "ident": ident, "identb": identb,
    }
    in_maps = []
    for b in range(B):
        m = dict(shared)
        m["y"] = np.ascontiguousarray(y[b])
        m["mem"] = np.ascontiguousarray(memory[b, start:start + W])
        in_maps.append(m)

    if "nc" not in _NC_CACHE:
        _NC_CACHE["nc"] = _build()
    nc = _NC_CACHE["nc"]

    res = run_bass_kernel_spmd(nc, in_maps, list(range(B)), trace=_trace)
    _NC_CACHE["last_res"] = res
    com_t_all = np.stack([res.results[b]["out_com"] for b in range(B)], axis=0)
    summary_dense = np.stack([res.results[b]["out_sum"] for b in range(B)], axis=0)
    return com_t_all.astype(np.float32), summary_dense.astype(np.float32)


# revision 26
# speedup vs baseline: 1.0022x; 1.0022x over previous
"""BarMamba sparse-attention kernel for Trainium2 (8 NeuronCores, data-parallel over batch).

Stage 1: block-diagonal bar attention (bar token attends to its 31 notes).
Stage 2: DSNT attention of bar summaries over memory level 2 (W=1024).

Design notes (per core, batch element b):
 - Everything is matmul-centric. Activations live in "transposed" layouts
   (D on partitions) so the PE can contract over D; fp32 data is bitcast to
   float32r for full-rate matmuls (free dim >= 256).
 - Stage-1 scores are computed per head as 4 column-tiled matmuls
   (32 bars each) over that group's 992 packed note-keys, with a
   block-diagonal -1e9 mask added via an extra identity matmul; exp on ACT
   produces unnormalized weights and their row sums (softmax denominators)
   in one instruction.
 - ctx = aw @ V runs on the PE via per-(head, group) accumulated matmuls
   with PE-transposed attention weights.
 - Softmax normalization is folded into the ctx PSUM eviction.
 - Stage-2 softmax over W=1024 is unnormalized-exp + fused dot with t_norm
   (scalar_tensor_tensor accumulate), normalized at the end.
"""

import math
from contextlib import ExitStack

import numpy as np
import ml_dtypes

import concourse.bass as bass
import concourse.bacc as bacc
import concourse.mybir as mybir
import concourse.tile as tile
from concourse.vector_clock import ScopedClock
from concourse.bass import SemaphoreHandle
from concourse.bass_utils import run_bass_kernel_spmd

B, S, D, H, P = 8, 4096, 512, 8, 32
NBAR = S // P        # 128 bars
DH = D // H          # 64
NOTE = P - 1         # 31 notes per bar
GRP = 32             # bars per score column-group
NGRP = NBAR // GRP   # 4
KEYS = GRP * P       # 1024 keys per group (bar token = masked dummy key)
NTILE = 128          # keys per 128-token window (4 bars x 32)
W = 1024             # level-2 width
LVL = 2
NEG = -1.0e9

F32 = mybir.dt.float32
F32R = mybir.dt.float32r
BF16 = mybir.dt.bfloat16
AX = mybir.AxisListType
ALU = mybir.AluOpType
ACTF = mybir.ActivationFunctionType


class PatchedTileContext(tile.TileContext):
    """TileContext whose exit drain splits sem waits into standalone wait
    instructions (this walrus build caps sync-waits per CTRL instruction)."""

    def _drain_and_barrier(self, tick_clock, wait_clock):
        nc = self.nc
        probe = mybir.InstDrain(name=f"I-probe-{nc.next_id()}")
        probe.engine = mybir.EngineType.SP
        wait_clock.add_sem_waits(probe, ScopedClock({None: tick_clock.global_clock}))
        si = probe.sync_info
        waits = list(si.on_wait) if si is not None else []
        for w in waits:
            sem = SemaphoreHandle(num=w.id, name=w.ant_name)
            nc.sync.wait_ge(sem, w.wait_value)
        nc.sync.drain()
        nc.all_engine_barrier()
        assert self.sems is not None
        popped = nc._tile_sem_poison_stack.pop()
        assert popped is self._sem_poison
        nc.clear_and_free_semaphores(list(self.sems.allocated().values()))
        nc.all_engine_barrier()


def r(ap):
    return ap.bitcast(F32R)


def _build():
    nc = bacc.Bacc()

    y_ext = nc.declare_dram_parameter("y", [S, D], F32R, isOutput=False)
    mem_ext = nc.declare_dram_parameter("mem", [W, D], F32R, isOutput=False)
    wnames = ["wq", "wk", "wv", "wo", "wq2", "wkm"]
    w_ext = {n: nc.declare_dram_parameter(n, [128, 4, D], F32R, isOutput=False)
             for n in wnames}
    peT_ext = nc.declare_dram_parameter("peT", [128, 4, NBAR], F32, isOutput=False)
    tpeT_ext = nc.declare_dram_parameter("tpeT", [128, 4, W], BF16, isOutput=False)
    tbc_ext = nc.declare_dram_parameter("tbc", [128, W], F32, isOutput=False)
    maskR_ext = nc.declare_dram_parameter("maskR", [128, KEYS], F32R, isOutput=False)
    id_ext = nc.declare_dram_parameter("ident", [128, 128], F32R, isOutput=False)
    idb_ext = nc.declare_dram_parameter("identb", [128, 128], BF16, isOutput=False)
    out_sum = nc.declare_dram_parameter("out_sum", [S, D], F32, isOutput=True)
    out_com = nc.declare_dram_parameter("out_com", [S, 1], F32, isOutput=True)

    with tile.TileContext(nc) as tc, ExitStack() as ctx:
        cpool = ctx.enter_context(tc.tile_pool(name="consts", bufs=1))
        small = ctx.enter_context(tc.tile_pool(name="small", bufs=1))
        pA = ctx.enter_context(tc.tile_pool(name="pA", bufs=3, space="PSUM"))
        pB = ctx.enter_context(tc.tile_pool(name="pB", bufs=2, space="PSUM"))
        pC = ctx.enter_context(tc.tile_pool(name="pC", bufs=1, space="PSUM"))

        # ---- constants / weights ----
        ident = cpool.tile([128, 128], F32R, tag="ident")
        nc.sync.dma_start(ident[:], id_ext[:])
        identb = cpool.tile([128, 128], BF16, tag="identb")
        nc.sync.dma_start(identb[:], idb_ext[:])
        peT = cpool.tile([128, 4, NBAR], F32, tag="peT")
        nc.sync.dma_start(peT[:], peT_ext[:])
        maskR = cpool.tile([128, KEYS], F32R, tag="maskR")
        wsb = {}
        for n in ["wq", "wk", "wv"]:
            wsb[n] = cpool.tile([128, 4, D], F32R, tag=f"w_{n}", name=f"w_{n}")

        denom1 = small.tile([128, H], F32, tag="denom1")
        denom2 = small.tile([128, H], F32, tag="denom2")
        ctraw = small.tile([128, H], F32, tag="ctraw")
        awt = small.tile([128, H, 8, NBAR], BF16, tag="awt")
        summary = small.tile([NBAR, D], F32R, tag="summary")
        com = small.tile([NBAR, 1], F32, tag="com")

        def transpose4(src, tag):
            """src: [128, 512] f32 sbuf -> psum [128, 4, 128] with
            pt[p, dt, c] = src[c, dt*128 + p]."""
            pt = pA.tile([128, 4, 128], F32, tag="pa")
            for dt in range(4):
                nc.tensor.transpose(r(pt[:, dt, :]), r(src[:, dt * 128:(dt + 1) * 128]),
                                    r(ident[:]))
            return pt

        vtp = ctx.enter_context(tc.tile_pool(name="vtp", bufs=1))
        ytp_ctx = tc.tile_pool(name="ytp", bufs=1, side="right")
        ytp = ytp_ctx.__enter__()
        with tc.tile_pool(name="stage1", bufs=1) as s1, \
             tc.tile_pool(name="yin", bufs=3) as yin, \
             tc.tile_pool(name="awp", bufs=2) as awp:

            yt = ytp.tile([128, 4, S], F32R, tag="yt")    # y_pe^T  (8 MiB)
            vt = vtp.tile([128, 32, D], BF16, tag="vt")   # V tiles (4 MiB)
            kt = s1.tile([128, 4, S], BF16, tag="kt")     # K^T     (4 MiB)

            # ---- phase A: load y, transpose(+bar PE), project V and K ----
            for t in range(32):
                ytile = yin.tile([128, D], F32R, tag="ytile")
                nc.sync.dma_start(ytile[:], y_ext[t * 128:(t + 1) * 128, :])
                if t == 0:
                    nc.sync.dma_start(wsb["wv"][:], w_ext["wv"][:])
                elif t == 1:
                    nc.sync.dma_start(wsb["wk"][:], w_ext["wk"][:])
                elif t == 8:
                    nc.sync.dma_start(wsb["wq"][:], w_ext["wq"][:])
                    nc.sync.dma_start(maskR[:], maskR_ext[:])
                pt = transpose4(ytile, "ptrans")
                o = yt[:, :, t * 128:(t + 1) * 128].rearrange(
                    "p dt (j q) -> p dt j q", q=P)
                i0 = pt[:].rearrange("p dt (j q) -> p dt j q", q=P)
                i1 = peT[:, :, 4 * t:4 * t + 4].broadcast_to((128, 4, 4, P))
                nc.vector.tensor_add(o, i0, i1)

                pv = pA.tile([128, 512], F32, tag="pa")
                for kti in range(4):
                    lhsT = yt[:, kti, t * 128:(t + 1) * 128]
                    nc.tensor.matmul(pv[:], r(lhsT), r(wsb["wv"][:, kti, :]),
                                     start=(kti == 0), stop=(kti == 3))
                if t % 2 == 0:
                    nc.scalar.copy(vt[:, t, :], pv[:])
                else:
                    nc.vector.tensor_copy(vt[:, t, :], pv[:])

                if t % 4 == 3:
                    c = t // 4
                    for mt in range(4):
                        pk = pA.tile([128, 512], F32, tag="pa")
                        for kti in range(4):
                            nc.tensor.matmul(
                                pk[:], r(wsb["wk"][:, kti, mt * 128:(mt + 1) * 128]),
                                r(yt[:, kti, c * 512:(c + 1) * 512]),
                                start=(kti == 0), stop=(kti == 3))
                        dst = kt[:, mt, c * 512:(c + 1) * 512]
                        if mt % 2 == 0:
                            nc.scalar.copy(dst, pk[:])
                        else:
                            nc.vector.tensor_copy(dst, pk[:])

            # ---- phase q: bar-token queries, then transpose ----
            pq = pA.tile([128, D], F32, tag="pa")
            for kti in range(4):
                bars = yt[:, kti, :].rearrange("p (n q) -> p n q", q=P)[:, :, 0:1]
                nc.tensor.matmul(pq[:], r(bars), r(wsb["wq"][:, kti, :]),
                                 start=(kti == 0), stop=(kti == 3))
            q_sb = small.tile([128, D], F32R, tag="scr512")
            nc.vector.tensor_copy(q_sb[:], pq[:])
            pt = transpose4(q_sb, "ptrans")
            qt = small.tile([128, 4, 128], BF16, tag="qt")
            nc.vector.tensor_copy(qt[:], pt[:])

            ytp_ctx.__exit__(None, None, None)

            # ---- prefetch stage-2 inputs + weights while scores run ----
            s2 = ctx.enter_context(tc.tile_pool(name="s2", bufs=1, side="right"))
            minp = ctx.enter_context(tc.tile_pool(name="minp", bufs=2, side="right"))
            late = ctx.enter_context(tc.tile_pool(name="late", bufs=1, side="right"))
            for n in ["wo", "wq2", "wkm"]:
                wsb[n] = late.tile([128, 4, D], F32R, tag=f"w_{n}", name=f"w_{n}")
                nc.sync.dma_start(wsb[n][:], w_ext[n][:])
            tpeT = s2.tile([128, 4, W], BF16, tag="tpeT")
            nc.sync.dma_start(tpeT[:], tpeT_ext[:])
            tbc = s2.tile([128, W], F32, tag="tbc")
            nc.sync.dma_start(tbc[:], tbc_ext[:])
            memT = s2.tile([128, 4, W], F32R, tag="memT")
            kmemT = s2.tile([128, 4, W], F32R, tag="kmemT")
            for t in range(8):
                mtile = minp.tile([128, D], F32R, tag="mtile")
                nc.sync.dma_start(mtile[:], mem_ext[t * 128:(t + 1) * 128, :])
                pt = transpose4(mtile, "ptrans")
                nc.vector.tensor_copy(memT[:, :, t * 128:(t + 1) * 128], pt[:])
            for mt in range(4):
                for wc in range(2):
                    pk = pA.tile([128, 512], F32, tag="pa")
                    for kti in range(4):
                        nc.tensor.matmul(
                            pk[:], r(wsb["wkm"][:, kti, mt * 128:(mt + 1) * 128]),
                            r(memT[:, kti, wc * 512:(wc + 1) * 512]),
                            start=(kti == 0), stop=(kti == 3))
                    nc.vector.tensor_add(
                        kmemT[:, mt, wc * 512:(wc + 1) * 512], pk[:],
                        tpeT[:, mt, wc * 512:(wc + 1) * 512])

            # ---- scores + exp + transposed weights, per head ----
            for h in range(H):
                mt = h // 2
                p0 = (h % 2) * DH
                ps = pB.tile([128, KEYS], F32, tag="pb")
                for wc in range(2):
                    nc.tensor.matmul(ps[:, wc * 512:(wc + 1) * 512],
                                     r(ident[:]),
                                     r(maskR[:, wc * 512:(wc + 1) * 512]),
                                     start=True, stop=False,
                                     skip_group_check=True)
                for g in range(NGRP):
                    lhsT = qt[p0:p0 + DH, mt, g * GRP:(g + 1) * GRP]
                    for wc in range(2):
                        rhs = kt[p0:p0 + DH, mt,
                                 g * KEYS + wc * 512:g * KEYS + (wc + 1) * 512]
                        nc.tensor.matmul(
                            ps[g * GRP:(g + 1) * GRP, wc * 512:(wc + 1) * 512],
                            lhsT, rhs,
                            start=False, stop=True,
                            skip_group_check=True,
                            tile_position=(p0, g * GRP))
                aw = awp.tile([128, KEYS], BF16, tag="aw")
                nc.scalar.activation(aw[:], ps[:], ACTF.Exp, bias=0.0,
                                     scale=0.125,
                                     accum_out=denom1[:, h:h + 1])
                paw = pB.tile([128, 8, 128], BF16, tag="pb")
                for ct in range(8):
                    nc.tensor.transpose(paw[:, ct, :],
                                        aw[:, ct * 128:(ct + 1) * 128],
                                        identb[:])
                nc.vector.tensor_copy(awt[:, h, :, :], paw[:])

        # ---- phase ctx: PE-accumulated attention readout ----
        with tc.tile_pool(name="ctxp", bufs=1) as ctxp:
            psc = pC.tile([128, D], F32, tag="pctx")
            for h in range(H):
                for g in range(NGRP):
                    for ct in range(8):
                        lhsT = awt[:, h, ct, g * GRP:(g + 1) * GRP]
                        rhs = vt[:, 8 * g + ct, h * DH:(h + 1) * DH]
                        nc.tensor.matmul(
                            psc[g * GRP:(g + 1) * GRP, h * DH:(h + 1) * DH],
                            lhsT, rhs, start=(ct == 0), stop=(ct == 7),
                            skip_group_check=True, tile_position=(0, g * GRP))
            recip1 = small.tile([128, H], F32, tag="recip1")
            nc.vector.reciprocal(recip1[:], denom1[:])
            ctxn = small.tile([128, D], F32R, tag="scr512")
            nc.vector.tensor_mul(
                ctxn[:].rearrange("p (h d) -> p h d", h=H),
                psc[:].rearrange("p (h d) -> p h d", h=H),
                recip1[:].broadcast_to((128, H, DH)))

            # ---- summary = ctxn @ Wo ----
            pt = transpose4(ctxn, "ptrans")
            ctxT = small.tile([128, 4, 128], F32R, tag="ctxT")
            nc.vector.tensor_copy(ctxT[:], pt[:])
            psu = pA.tile([128, D], F32, tag="pa")
            for kt in range(4):
                nc.tensor.matmul(psu[:], r(ctxT[:, kt, :]), r(wsb["wo"][:, kt, :]),
                                 start=(kt == 0), stop=(kt == 3))
            nc.vector.tensor_copy(summary[:], psu[:])
            z32 = small.tile([32, D], F32, tag="z32")
            nc.gpsimd.memset(z32[:], 0.0)
            nc.sync.dma_start(out_sum[0:P, :], z32[:])
            nc.sync.dma_start(
                out_sum[P:S, :].rearrange("(j q) d -> j q d", q=P),
                summary[0:NBAR - 1, :].bitcast(F32).unsqueeze(1)
                .broadcast_to((NBAR - 1, P, D)))

            # ---- Q2 = summary @ Wq2, then transpose ----
            pt = transpose4(summary, "ptrans")
            sumT = small.tile([128, 4, 128], F32R, tag="sumT")
            nc.vector.tensor_copy(sumT[:], pt[:])
            pq2 = pA.tile([128, D], F32, tag="pa")
            for kt in range(4):
                nc.tensor.matmul(pq2[:], r(sumT[:, kt, :]), r(wsb["wq2"][:, kt, :]),
                                 start=(kt == 0), stop=(kt == 3))
            q2_sb = small.tile([128, D], F32R, tag="scr512")
            nc.vector.tensor_copy(q2_sb[:], pq2[:])
            pt = transpose4(q2_sb, "ptrans")
            q2t = small.tile([128, 4, 128], F32R, tag="q2t")
            nc.vector.tensor_copy(q2t[:], pt[:])

        # ---- stage 2: memory attention ----
        with tc.tile_pool(name="e2p", bufs=2) as e2p:
            for h in range(H):
                p0 = (h % 2) * DH
                mt = h // 2
                ps2 = pB.tile([128, W], F32, tag="pb")
                for wc in range(2):
                    nc.tensor.matmul(
                        ps2[:, wc * 512:(wc + 1) * 512],
                        r(q2t[p0:p0 + DH, mt, :]),
                        r(kmemT[p0:p0 + DH, mt, wc * 512:(wc + 1) * 512]),
                        start=True, stop=True)
                e2 = e2p.tile([128, W], F32, tag="e2")
                nc.scalar.activation(e2[:], ps2[:], ACTF.Exp, bias=0.0, scale=0.125,
                                     accum_out=denom2[:, h:h + 1])
                scr = e2p.tile([128, W], F32, tag="scr")
                nc.vector.scalar_tensor_tensor(
                    out=scr[:], in0=e2[:], scalar=1.0, in1=tbc[:],
                    op0=ALU.mult, op1=ALU.mult,
                    accum_out=ctraw[:, h:h + 1])

            # com = (1/H) * sum_h ctraw/denom2
            recip2 = small.tile([128, H], F32, tag="recip2")
            nc.vector.reciprocal(recip2[:], denom2[:])
            tmp8 = small.tile([128, H], F32, tag="tmp8")
            nc.vector.tensor_mul(tmp8[:], ctraw[:], recip2[:])
            nc.vector.tensor_reduce(com[:], tmp8[:], axis=AX.X, op=ALU.add)
            nc.scalar.mul(com[:], com[:], 1.0 / H)

        # ---- com output ----
        with tc.tile_pool(name="outp", bufs=1) as outp:
            T = outp.tile([128, P], F32, tag="Tcom")
            nc.gpsimd.memset(T[:], 0.0)
            nc.sync.dma_start(T[1:128, 0:1], com[0:NBAR - 1, 0:1])
            nc.sync.dma_start(out_com[:].rearrange("(j q) o -> j q o", q=P),
                              T[:].unsqueeze(-1))
    nc.compile()
    return nc


def _sin_pe_np(pos, d):
    half = d // 2
    inv_freq = (1.0 / (10000.0 ** (np.arange(half, dtype=np.float32) / half))
                ).astype(np.float32)
    ang = pos.astype(np.float32)[:, None] * inv_freq[None, :]
    return np.concatenate([np.sin(ang), np.cos(ang)], axis=-1).astype(np.float32)


_NC_CACHE = {}


def kernel(y, memory, spatial_shapes, level_start_index, bar_mask,
           Wq_bar, Wk_bar, Wv_bar, Wo_bar, Wq2, Wk_mem, _trace=False):
    y = np.asarray(y)
    memory = np.asarray(memory)
    bar_mask = np.asarray(bar_mask)
    start = int(np.asarray(level_start_index)[LVL])
    w_l = int(np.asarray(spatial_shapes)[LVL, 1])
    assert w_l == W and y.shape == (B, S, D)

    # bar PE table (bar_mask is periodic with period P; PE constant per block)
    bidx = np.cumsum(bar_mask[0].astype(np.float32))[::P]          # [128]
    peJ = _sin_pe_np(bidx, D)                                      # [128, 512]
    peT = np.ascontiguousarray(peJ.reshape(NBAR, 4, 128).transpose(2, 1, 0))

    t_norm = (np.arange(W, dtype=np.float32) / max(W - 1, 1)).astype(np.float32)
    time_pe = _sin_pe_np(t_norm * W, D)                            # [1024, 512]
    tpeT = np.ascontiguousarray(time_pe.reshape(W, 4, 128).transpose(2, 1, 0)).astype(ml_dtypes.bfloat16)
    tbc = np.ascontiguousarray(np.broadcast_to(t_norm, (128, W)))

    key_idx = np.arange(KEYS)
    key_grp = key_idx // P
    key_k = key_idx % P
    diag = (key_grp[None, :] == (np.arange(128)[:, None] % GRP)) & (key_k[None, :] > 0)
    maskR = np.where(diag, 0.0, NEG).astype(np.float32)
    ident = np.eye(128, dtype=np.float32)
    identb = np.eye(128).astype(ml_dtypes.bfloat16)

    def wprep(w):
        return np.ascontiguousarray(
            np.asarray(w, dtype=np.float32).reshape(4, 128, D).transpose(1, 0, 2))

    shared = {
        "wq": wprep(Wq_bar), "wk": wprep(Wk_bar), "wv": wprep(Wv_bar),
        "wo": wprep(Wo_bar), "wq2": wprep(Wq2), "wkm": wprep(Wk_mem),
        "peT": peT, "tpeT": tpeT, "tbc": tbc, "maskR": maskR,
        "ident": ident, "identb": identb,
    }
    in_maps = []
    for b in range(B):
        m = dict(shared)
        m["y"] = np.ascontiguousarray(y[b])
        m["mem"] = np.ascontiguousarray(memory[b, start:start + W])
        in_maps.append(m)

    if "nc" not in _NC_CACHE:
        _NC_CACHE["nc"] = _build()
    nc = _NC_CACHE["nc"]

    res = run_bass_kernel_spmd(nc, in_maps, list(range(B)), trace=_trace)
    _NC_CACHE["last_res"] = res
    com_t_all = np.stack([res.results[b]["out_com"] for b in range(B)], axis=0)
    summary_dense = np.stack([res.results[b]["out_sum"] for b in range(B)], axis=0)
    return com_t_all.astype(np.float32), summary_dense.astype(np.float32)


# revision 29
# speedup vs baseline: 1.0112x; 1.0089x over previous
"""BarMamba sparse-attention kernel for Trainium2 (8 NeuronCores, data-parallel over batch).

Stage 1: block-diagonal bar attention (bar token attends to its 31 notes).
Stage 2: DSNT attention of bar summaries over memory level 2 (W=1024).

Design notes (per core, batch element b):
 - Everything is matmul-centric. Activations live in "transposed" layouts
   (D on partitions) so the PE can contract over D; fp32 data is bitcast to
   float32r for full-rate matmuls (free dim >= 256).
 - Stage-1 scores are computed per head as 4 column-tiled matmuls
   (32 bars each) over that group's 992 packed note-keys, with a
   block-diagonal -1e9 mask added via an extra identity matmul; exp on ACT
   produces unnormalized weights and their row sums (softmax denominators)
   in one instruction.
 - ctx = aw @ V runs on the PE via per-(head, group) accumulated matmuls
   with PE-transposed attention weights.
 - Softmax normalization is folded into the ctx PSUM eviction.
 - Stage-2 softmax over W=1024 is unnormalized-exp + fused dot with t_norm
   (scalar_tensor_tensor accumulate), normalized at the end.
"""

import math
from contextlib import ExitStack

import numpy as np
import ml_dtypes

import concourse.bass as bass
import concourse.bacc as bacc
import concourse.mybir as mybir
import concourse.tile as tile
from concourse.vector_clock import ScopedClock
from concourse.bass import SemaphoreHandle
from concourse.bass_utils import run_bass_kernel_spmd

B, S, D, H, P = 8, 4096, 512, 8, 32
NBAR = S // P        # 128 bars
DH = D // H          # 64
NOTE = P - 1         # 31 notes per bar
GRP = 32             # bars per score column-group
NGRP = NBAR // GRP   # 4
KEYS = GRP * P       # 1024 keys per group (bar token = masked dummy key)
NTILE = 128          # keys per 128-token window (4 bars x 32)
W = 1024             # level-2 width
LVL = 2
NEG = -1.0e9

F32 = mybir.dt.float32
F32R = mybir.dt.float32r
BF16 = mybir.dt.bfloat16
AX = mybir.AxisListType
ALU = mybir.AluOpType
ACTF = mybir.ActivationFunctionType


class PatchedTileContext(tile.TileContext):
    """TileContext whose exit drain splits sem waits into standalone wait
    instructions (this walrus build caps sync-waits per CTRL instruction)."""

    def _drain_and_barrier(self, tick_clock, wait_clock):
        nc = self.nc
        probe = mybir.InstDrain(name=f"I-probe-{nc.next_id()}")
        probe.engine = mybir.EngineType.SP
        wait_clock.add_sem_waits(probe, ScopedClock({None: tick_clock.global_clock}))
        si = probe.sync_info
        waits = list(si.on_wait) if si is not None else []
        for w in waits:
            sem = SemaphoreHandle(num=w.id, name=w.ant_name)
            nc.sync.wait_ge(sem, w.wait_value)
        nc.sync.drain()
        nc.all_engine_barrier()
        assert self.sems is not None
        popped = nc._tile_sem_poison_stack.pop()
        assert popped is self._sem_poison
        nc.clear_and_free_semaphores(list(self.sems.allocated().values()))
        nc.all_engine_barrier()


def r(ap):
    return ap.bitcast(F32R)


def _build():
    nc = bacc.Bacc()

    y_ext = nc.declare_dram_parameter("y", [S, D], F32R, isOutput=False)
    mem_ext = nc.declare_dram_parameter("mem", [W, D], F32R, isOutput=False)
    wnames = ["wq", "wk", "wv", "wo", "wq2", "wkm"]
    w_ext = {n: nc.declare_dram_parameter(n, [128, 4, D], F32R, isOutput=False)
             for n in wnames}
    peT_ext = nc.declare_dram_parameter("peT", [128, 4, NBAR], F32, isOutput=False)
    tpeT_ext = nc.declare_dram_parameter("tpeT", [128, 4, W], BF16, isOutput=False)
    tbc_ext = nc.declare_dram_parameter("tbc", [128, W], F32, isOutput=False)
    maskR_ext = nc.declare_dram_parameter("maskR", [128, KEYS], F32R, isOutput=False)
    id_ext = nc.declare_dram_parameter("ident", [128, 128], F32R, isOutput=False)
    idb_ext = nc.declare_dram_parameter("identb", [128, 128], BF16, isOutput=False)
    out_sum = nc.declare_dram_parameter("out_sum", [S, D], F32, isOutput=True)
    out_com = nc.declare_dram_parameter("out_com", [S, 1], F32, isOutput=True)

    with tile.TileContext(nc) as tc, ExitStack() as ctx:
        cpool = ctx.enter_context(tc.tile_pool(name="consts", bufs=1))
        small = ctx.enter_context(tc.tile_pool(name="small", bufs=1))
        pA = ctx.enter_context(tc.tile_pool(name="pA", bufs=3, space="PSUM"))
        pB = ctx.enter_context(tc.tile_pool(name="pB", bufs=2, space="PSUM"))
        pC = ctx.enter_context(tc.tile_pool(name="pC", bufs=1, space="PSUM"))

        # ---- constants / weights ----
        ident = cpool.tile([128, 128], F32R, tag="ident")
        nc.sync.dma_start(ident[:], id_ext[:])
        identb = cpool.tile([128, 128], BF16, tag="identb")
        nc.sync.dma_start(identb[:], idb_ext[:])
        peT = cpool.tile([128, 4, NBAR], F32, tag="peT")
        nc.sync.dma_start(peT[:], peT_ext[:])
        maskR = cpool.tile([128, KEYS], F32R, tag="maskR")
        wsb = {}
        for n in ["wq", "wk", "wv"]:
            wsb[n] = cpool.tile([128, 4, D], F32R, tag=f"w_{n}", name=f"w_{n}")

        denom1 = small.tile([128, H], F32, tag="denom1")
        denom2 = small.tile([128, H], F32, tag="denom2")
        ctraw = small.tile([128, H], F32, tag="ctraw")
        awt = small.tile([128, H, 8, NBAR], BF16, tag="awt")
        summary = small.tile([NBAR, D], F32R, tag="summary")
        com = small.tile([NBAR, 1], F32, tag="com")

        def transpose4(src, tag):
            """src: [128, 512] f32 sbuf -> psum [128, 4, 128] with
            pt[p, dt, c] = src[c, dt*128 + p]."""
            pt = pA.tile([128, 4, 128], F32, tag="pa")
            for dt in range(4):
                nc.tensor.transpose(r(pt[:, dt, :]), r(src[:, dt * 128:(dt + 1) * 128]),
                                    r(ident[:]))
            return pt

        vtp = ctx.enter_context(tc.tile_pool(name="vtp", bufs=1))
        ytp_ctx = tc.tile_pool(name="ytp", bufs=1, side="right")
        ytp = ytp_ctx.__enter__()
        with tc.tile_pool(name="stage1", bufs=1) as s1, \
             tc.tile_pool(name="yin", bufs=3) as yin, \
             tc.tile_pool(name="awp", bufs=2) as awp:

            yt = ytp.tile([128, 4, S], F32R, tag="yt")    # y_pe^T  (8 MiB)
            vt = vtp.tile([128, 32, D], BF16, tag="vt")   # V tiles (4 MiB)
            kt = s1.tile([128, 4, S], BF16, tag="kt")     # K^T     (4 MiB)

            # ---- phase A: load y, transpose(+bar PE), project V and K ----
            for t in range(32):
                ytile = yin.tile([128, D], F32R, tag="ytile")
                yeng = nc.sync if t % 2 == 0 else nc.scalar
                yeng.dma_start(ytile[:], y_ext[t * 128:(t + 1) * 128, :])
                if t == 0:
                    nc.sync.dma_start(wsb["wv"][:], w_ext["wv"][:])
                elif t == 1:
                    nc.sync.dma_start(wsb["wk"][:], w_ext["wk"][:])
                elif t == 8:
                    nc.sync.dma_start(wsb["wq"][:], w_ext["wq"][:])
                    nc.sync.dma_start(maskR[:], maskR_ext[:])
                pt = transpose4(ytile, "ptrans")
                o = yt[:, :, t * 128:(t + 1) * 128].rearrange(
                    "p dt (j q) -> p dt j q", q=P)
                i0 = pt[:].rearrange("p dt (j q) -> p dt j q", q=P)
                i1 = peT[:, :, 4 * t:4 * t + 4].broadcast_to((128, 4, 4, P))
                nc.vector.tensor_add(o, i0, i1)

                pv = pA.tile([128, 512], F32, tag="pa")
                for kti in range(4):
                    lhsT = yt[:, kti, t * 128:(t + 1) * 128]
                    nc.tensor.matmul(pv[:], r(lhsT), r(wsb["wv"][:, kti, :]),
                                     start=(kti == 0), stop=(kti == 3))
                if t % 2 == 0:
                    nc.scalar.copy(vt[:, t, :], pv[:])
                else:
                    nc.vector.tensor_copy(vt[:, t, :], pv[:])

                if t % 4 == 3:
                    c = t // 4
                    for mt in range(4):
                        pk = pA.tile([128, 512], F32, tag="pa")
                        for kti in range(4):
                            nc.tensor.matmul(
                                pk[:], r(wsb["wk"][:, kti, mt * 128:(mt + 1) * 128]),
                                r(yt[:, kti, c * 512:(c + 1) * 512]),
                                start=(kti == 0), stop=(kti == 3))
                        dst = kt[:, mt, c * 512:(c + 1) * 512]
                        if mt % 2 == 0:
                            nc.scalar.copy(dst, pk[:])
                        else:
                            nc.vector.tensor_copy(dst, pk[:])

            # ---- phase q: bar-token queries, then transpose ----
            pq = pA.tile([128, D], F32, tag="pa")
            for kti in range(4):
                bars = yt[:, kti, :].rearrange("p (n q) -> p n q", q=P)[:, :, 0:1]
                nc.tensor.matmul(pq[:], r(bars), r(wsb["wq"][:, kti, :]),
                                 start=(kti == 0), stop=(kti == 3))
            q_sb = small.tile([128, D], F32R, tag="scr512")
            nc.vector.tensor_copy(q_sb[:], pq[:])
            pt = transpose4(q_sb, "ptrans")
            qt = small.tile([128, 4, 128], BF16, tag="qt")
            nc.vector.tensor_copy(qt[:], pt[:])

            ytp_ctx.__exit__(None, None, None)

            # ---- prefetch stage-2 inputs + weights while scores run ----
            s2 = ctx.enter_context(tc.tile_pool(name="s2", bufs=1, side="right"))
            minp = ctx.enter_context(tc.tile_pool(name="minp", bufs=2, side="right"))
            late = ctx.enter_context(tc.tile_pool(name="late", bufs=1, side="right"))
            for n in ["wo", "wq2", "wkm"]:
                wsb[n] = late.tile([128, 4, D], F32R, tag=f"w_{n}", name=f"w_{n}")
                nc.sync.dma_start(wsb[n][:], w_ext[n][:])
            tpeT = s2.tile([128, 4, W], BF16, tag="tpeT")
            nc.sync.dma_start(tpeT[:], tpeT_ext[:])
            tbc = s2.tile([128, W], F32, tag="tbc")
            nc.sync.dma_start(tbc[:], tbc_ext[:])
            memT = s2.tile([128, 4, W], F32R, tag="memT")
            kmemT = s2.tile([128, 4, W], F32R, tag="kmemT")
            for t in range(8):
                mtile = minp.tile([128, D], F32R, tag="mtile")
                nc.sync.dma_start(mtile[:], mem_ext[t * 128:(t + 1) * 128, :])
                pt = transpose4(mtile, "ptrans")
                nc.vector.tensor_copy(memT[:, :, t * 128:(t + 1) * 128], pt[:])
            for mt in range(4):
                for wc in range(2):
                    pk = pA.tile([128, 512], F32, tag="pa")
                    for kti in range(4):
                        nc.tensor.matmul(
                            pk[:], r(wsb["wkm"][:, kti, mt * 128:(mt + 1) * 128]),
                            r(memT[:, kti, wc * 512:(wc + 1) * 512]),
                            start=(kti == 0), stop=(kti == 3))
                    nc.vector.tensor_add(
                        kmemT[:, mt, wc * 512:(wc + 1) * 512], pk[:],
                        tpeT[:, mt, wc * 512:(wc + 1) * 512])

            # ---- scores + exp + transposed weights, per head ----
            for h in range(H):
                mt = h // 2
                p0 = (h % 2) * DH
                ps = pB.tile([128, KEYS], F32, tag="pb")
                for wc in range(2):
                    nc.tensor.matmul(ps[:, wc * 512:(wc + 1) * 512],
                                     r(ident[:]),
                                     r(maskR[:, wc * 512:(wc + 1) * 512]),
                                     start=True, stop=False,
                                     skip_group_check=True)
                for g in range(NGRP):
                    lhsT = qt[p0:p0 + DH, mt, g * GRP:(g + 1) * GRP]
                    for wc in range(2):
                        rhs = kt[p0:p0 + DH, mt,
                                 g * KEYS + wc * 512:g * KEYS + (wc + 1) * 512]
                        nc.tensor.matmul(
                            ps[g * GRP:(g + 1) * GRP, wc * 512:(wc + 1) * 512],
                            lhsT, rhs,
                            start=False, stop=True,
                            skip_group_check=True,
                            tile_position=(p0, g * GRP))
                aw = awp.tile([128, KEYS], BF16, tag="aw")
                nc.scalar.activation(aw[:], ps[:], ACTF.Exp, bias=0.0,
                                     scale=0.125,
                                     accum_out=denom1[:, h:h + 1])
                paw = pB.tile([128, 8, 128], BF16, tag="pb")
                for ct in range(8):
                    nc.tensor.transpose(paw[:, ct, :],
                                        aw[:, ct * 128:(ct + 1) * 128],
                                        identb[:])
                nc.vector.tensor_copy(awt[:, h, :, :], paw[:])

        # ---- phase ctx: PE-accumulated attention readout ----
        with tc.tile_pool(name="ctxp", bufs=1) as ctxp:
            psc = pC.tile([128, D], F32, tag="pctx")
            for h in range(H):
                for g in range(NGRP):
                    for ct in range(8):
                        lhsT = awt[:, h, ct, g * GRP:(g + 1) * GRP]
                        rhs = vt[:, 8 * g + ct, h * DH:(h + 1) * DH]
                        nc.tensor.matmul(
                            psc[g * GRP:(g + 1) * GRP, h * DH:(h + 1) * DH],
                            lhsT, rhs, start=(ct == 0), stop=(ct == 7),
                            skip_group_check=True, tile_position=(0, g * GRP))
            recip1 = small.tile([128, H], F32, tag="recip1")
            nc.vector.reciprocal(recip1[:], denom1[:])
            ctxn = small.tile([128, D], F32R, tag="scr512")
            nc.vector.tensor_mul(
                ctxn[:].rearrange("p (h d) -> p h d", h=H),
                psc[:].rearrange("p (h d) -> p h d", h=H),
                recip1[:].broadcast_to((128, H, DH)))

            # ---- summary = ctxn @ Wo ----
            pt = transpose4(ctxn, "ptrans")
            ctxT = small.tile([128, 4, 128], F32R, tag="ctxT")
            nc.vector.tensor_copy(ctxT[:], pt[:])
            psu = pA.tile([128, D], F32, tag="pa")
            for kt in range(4):
                nc.tensor.matmul(psu[:], r(ctxT[:, kt, :]), r(wsb["wo"][:, kt, :]),
                                 start=(kt == 0), stop=(kt == 3))
            nc.vector.tensor_copy(summary[:], psu[:])
            z32 = small.tile([32, D], F32, tag="z32")
            nc.gpsimd.memset(z32[:], 0.0)
            nc.sync.dma_start(out_sum[0:P, :], z32[:])
            nc.sync.dma_start(
                out_sum[P:S, :].rearrange("(j q) d -> j q d", q=P),
                summary[0:NBAR - 1, :].bitcast(F32).unsqueeze(1)
                .broadcast_to((NBAR - 1, P, D)))

            # ---- Q2 = summary @ Wq2, then transpose ----
            pt = transpose4(summary, "ptrans")
            sumT = small.tile([128, 4, 128], F32R, tag="sumT")
            nc.vector.tensor_copy(sumT[:], pt[:])
            pq2 = pA.tile([128, D], F32, tag="pa")
            for kt in range(4):
                nc.tensor.matmul(pq2[:], r(sumT[:, kt, :]), r(wsb["wq2"][:, kt, :]),
                                 start=(kt == 0), stop=(kt == 3))
            q2_sb = small.tile([128, D], F32R, tag="scr512")
            nc.vector.tensor_copy(q2_sb[:], pq2[:])
            pt = transpose4(q2_sb, "ptrans")
            q2t = small.tile([128, 4, 128], F32R, tag="q2t")
            nc.vector.tensor_copy(q2t[:], pt[:])

        # ---- stage 2: memory attention ----
        with tc.tile_pool(name="e2p", bufs=2) as e2p:
            for h in range(H):
                p0 = (h % 2) * DH
                mt = h // 2
                ps2 = pB.tile([128, W], F32, tag="pb")
                for wc in range(2):
                    nc.tensor.matmul(
                        ps2[:, wc * 512:(wc + 1) * 512],
                        r(q2t[p0:p0 + DH, mt, :]),
                        r(kmemT[p0:p0 + DH, mt, wc * 512:(wc + 1) * 512]),
                        start=True, stop=True)
                e2 = e2p.tile([128, W], F32, tag="e2")
                nc.scalar.activation(e2[:], ps2[:], ACTF.Exp, bias=0.0, scale=0.125,
                                     accum_out=denom2[:, h:h + 1])
                scr = e2p.tile([128, W], F32, tag="scr")
                nc.vector.scalar_tensor_tensor(
                    out=scr[:], in0=e2[:], scalar=1.0, in1=tbc[:],
                    op0=ALU.mult, op1=ALU.mult,
                    accum_out=ctraw[:, h:h + 1])

            # com = (1/H) * sum_h ctraw/denom2
            recip2 = small.tile([128, H], F32, tag="recip2")
            nc.vector.reciprocal(recip2[:], denom2[:])
            tmp8 = small.tile([128, H], F32, tag="tmp8")
            nc.vector.tensor_mul(tmp8[:], ctraw[:], recip2[:])
            nc.vector.tensor_reduce(com[:], tmp8[:], axis=AX.X, op=ALU.add)
            nc.scalar.mul(com[:], com[:], 1.0 / H)

        # ---- com output ----
        with tc.tile_pool(name="outp", bufs=1) as outp:
            T = outp.tile([128, P], F32, tag="Tcom")
            nc.gpsimd.memset(T[:], 0.0)
            nc.sync.dma_start(T[1:128, 0:1], com[0:NBAR - 1, 0:1])
            nc.sync.dma_start(out_com[:].rearrange("(j q) o -> j q o", q=P),
                              T[:].unsqueeze(-1))
    nc.compile()
    return nc


def _sin_pe_np(pos, d):
    half = d // 2
    inv_freq = (1.0 / (10000.0 ** (np.arange(half, dtype=np.float32) / half))
                ).astype(np.float32)
    ang = pos.astype(np.float32)[:, None] * inv_freq[None, :]
    return np.concatenate([np.sin(ang), np.cos(ang)], axis=-1).astype(np.float32)


_NC_CACHE = {}


def kernel(y, memory, spatial_shapes, level_start_index, bar_mask,
           Wq_bar, Wk_bar, Wv_bar, Wo_bar, Wq2, Wk_mem, _trace=False):
    y = np.asarray(y)
    memory = np.asarray(memory)
    bar_mask = np.asarray(bar_mask)
    start = int(np.asarray(level_start_index)[LVL])
    w_l = int(np.asarray(spatial_shapes)[LVL, 1])
    assert w_l == W and y.shape == (B, S, D)

    # bar PE table (bar_mask is periodic with period P; PE constant per block)
    bidx = np.cumsum(bar_mask[0].astype(np.float32))[::P]          # [128]
    peJ = _sin_pe_np(bidx, D)                                      # [128, 512]
    peT = np.ascontiguousarray(peJ.reshape(NBAR, 4, 128).transpose(2, 1, 0))

    t_norm = (np.arange(W, dtype=np.float32) / max(W - 1, 1)).astype(np.float32)
    time_pe = _sin_pe_np(t_norm * W, D)                            # [1024, 512]
    tpeT = np.ascontiguousarray(time_pe.reshape(W, 4, 128).transpose(2, 1, 0)).astype(ml_dtypes.bfloat16)
    tbc = np.ascontiguousarray(np.broadcast_to(t_norm, (128, W)))

    key_idx = np.arange(KEYS)
    key_grp = key_idx // P
    key_k = key_idx % P
    diag = (key_grp[None, :] == (np.arange(128)[:, None] % GRP)) & (key_k[None, :] > 0)
    maskR = np.where(diag, 0.0, NEG).astype(np.float32)
    ident = np.eye(128, dtype=np.float32)
    identb = np.eye(128).astype(ml_dtypes.bfloat16)

    def wprep(w):
        return np.ascontiguousarray(
            np.asarray(w, dtype=np.float32).reshape(4, 128, D).transpose(1, 0, 2))

    shared = {
        "wq": wprep(Wq_bar), "wk": wprep(Wk_bar), "wv": wprep(Wv_bar),
        "wo": wprep(Wo_bar), "wq2": wprep(Wq2), "wkm": wprep(Wk_mem),
        "peT": peT, "tpeT": tpeT, "tbc": tbc, "maskR": maskR,
        "ident": ident, "identb": identb,
    }
    in_maps = []
    for b in range(B):
        m = dict(shared)
        m["y"] = np.ascontiguousarray(y[b])
        m["mem"] = np.ascontiguousarray(memory[b, start:start + W])
        in_maps.append(m)

    if "nc" not in _NC_CACHE:
        _NC_CACHE["nc"] = _build()
    nc = _NC_CACHE["nc"]

    res = run_bass_kernel_spmd(nc, in_maps, list(range(B)), trace=_trace)
    _NC_CACHE["last_res"] = res
    com_t_all = np.stack([res.results[b]["out_com"] for b in range(B)], axis=0)
    summary_dense = np.stack([res.results[b]["out_sum"] for b in range(B)], axis=0)
    return com_t_all.astype(np.float32), summary_dense.astype(np.float32)
